# revision 4
# baseline (speedup 1.0000x reference)
"""MAE self-attention (sparse_attention) Trainium2 Bass kernel.

Sharding: 8 cores = batch(2) x head-groups(4 groups of 3 heads).
Each core computes, for its (batch, 3 heads):
  - kv projection k^T = (Wk^T x^T), v = x @ Wv (x = [embx; hidden], keys padded
    2049 -> 2176 = 17*128)
  - scores^T[j, q] = k^T . q  (keys on partitions, queries on free dim)
  - p = exp(scale*scores + keybias[j])  (ACT, per-partition bias; masked/pad
    keys get bias -10000 -> exp underflows to exactly 0)
  - diagonal (q == j-1) zeroed via a small 128-wide mask multiply
  - out^T[d, q] = sum_j v'[j, d] p[j, q] with v' = [v | 1]; the extra ones
    column yields the softmax denominator in row 64.
Host divides by the denominator, transposes, and reassembles the full output.
"""

import numpy as np

import concourse.bacc as bacc
import concourse.bass as bass  # noqa: F401
import concourse.mybir as mybir
import concourse.tile as tile
from concourse.bass_utils import run_bass_kernel_spmd

F32 = mybir.dt.float32

B = 2
S = 2048          # queries
HID = 768
H = 12
D = 64
G = 3             # heads per core
NCORE = 8
SK = 2176         # padded key count (17 * 128); true keys = 2049
KC = HID // 128   # 6 contraction chunks
NT = SK // 128    # 17 key tiles
NEG = -10000.0
SCALE = 0.125     # D ** -0.5

Exp = mybir.ActivationFunctionType.Exp


def _build_nc():
    nc = bacc.Bacc(None, target_bir_lowering=False)

    xT_d = nc.dram_tensor("xT", [HID, SK], F32, kind="ExternalInput")
    qT_d = nc.dram_tensor("qT", [G * D, S], F32, kind="ExternalInput")
    w_d = nc.dram_tensor("W", [HID, 2 * G * D], F32, kind="ExternalInput")
    bk_d = nc.dram_tensor("bk", [D, G], F32, kind="ExternalInput")
    bv_d = nc.dram_tensor("bv", [128, G * D], F32, kind="ExternalInput")
    kb_d = nc.dram_tensor("kb", [128, NT], F32, kind="ExternalInput")
    dm0_d = nc.dram_tensor("dm0", [128, 128], F32, kind="ExternalInput")
    dmg_d = nc.dram_tensor("dmG", [128, 128], F32, kind="ExternalInput")
    out_d = nc.dram_tensor("outT", [G, D + 1, S], F32, kind="ExternalOutput")

    with tile.TileContext(nc) as tc:
        with (
            tc.tile_pool(name="const", bufs=1) as cpool,
            tc.tile_pool(name="work", bufs=3) as wpool,
            tc.tile_pool(name="ovec", bufs=2) as opool,
            tc.tile_pool(name="psA", bufs=2, space="PSUM") as psa,
            tc.tile_pool(name="psV", bufs=2, space="PSUM") as psv,
        ):
            xT_sb = cpool.tile([128, KC, SK], F32)
            w_sb = cpool.tile([128, KC, 2 * G * D], F32)
            qT_sb = cpool.tile([D, G, S], F32)
            bk_sb = cpool.tile([D, G], F32)
            bv_sb = cpool.tile([128, G * D], F32)
            kb_sb = cpool.tile([128, NT], F32)
            dm0_sb = cpool.tile([128, 128], F32)
            dmg_sb = cpool.tile([128, 128], F32)
            kT_sb = cpool.tile([D, G, SK], F32)
            v_sb = cpool.tile([128, NT, G, D + 1], F32)

            for kc in range(KC):
                nc.sync.dma_start(
                    out=w_sb[:, kc, :], in_=w_d[kc * 128 : (kc + 1) * 128, :]
                )
            for kc in range(KC):
                nc.sync.dma_start(
                    out=xT_sb[:, kc, :], in_=xT_d[kc * 128 : (kc + 1) * 128, :]
                )
            for h in range(G):
                nc.sync.dma_start(
                    out=qT_sb[:, h, :], in_=qT_d[h * D : (h + 1) * D, :]
                )
            nc.sync.dma_start(out=bk_sb, in_=bk_d[:, :])
            nc.sync.dma_start(out=bv_sb, in_=bv_d[:, :])
            nc.sync.dma_start(out=kb_sb, in_=kb_d[:, :])
            nc.sync.dma_start(out=dm0_sb, in_=dm0_d[:, :])
            nc.sync.dma_start(out=dmg_sb, in_=dmg_d[:, :])

            # ---- kv projection ----
            # k^T[c, t] per head: lhsT = Wk[:, 64h:64h+64], rhs = x^T chunks.
            for h in range(G):
                for c0 in range(0, SK, 1024):
                    csz = min(1024, SK - c0)
                    ps = psa.tile([128, 1024], F32, tag="ps")
                    for kc in range(KC):
                        for nn in range(0, csz, 512):
                            nsz = min(512, csz - nn)
                            nc.tensor.matmul(
                                ps[0:D, nn : nn + nsz],
                                w_sb[:, kc, h * D : (h + 1) * D],
                                xT_sb[:, kc, c0 + nn : c0 + nn + nsz],
                                start=(kc == 0),
                                stop=(kc == KC - 1),
                            )
                    nc.vector.tensor_scalar_add(
                        kT_sb[0:D, h, c0 : c0 + csz],
                        ps[0:D, 0:csz],
                        bk_sb[0:D, h : h + 1],
                    )

            # v[t, c]: lhsT = x^T[:, t-tile], rhs = Wv.
            for t in range(NT):
                ps = psa.tile([128, 1024], F32, tag="ps")
                for kc in range(KC):
                    nc.tensor.matmul(
                        ps[:, 0 : G * D],
                        xT_sb[:, kc, t * 128 : (t + 1) * 128],
                        w_sb[:, kc, G * D : 2 * G * D],
                        start=(kc == 0),
                        stop=(kc == KC - 1),
                    )
                nc.vector.tensor_add(
                    v_sb[:, t, :, 0:D],
                    ps[:, 0 : G * D].rearrange("p (h d) -> p h d", h=G),
                    bv_sb.rearrange("p (h d) -> p h d", h=G),
                )
                nc.vector.memset(v_sb[:, t, :, D : D + 1], 1.0)

            # ---- attention ----
            for h in range(G):
                for half in range(2):
                    q0 = half * 1024
                    pv = psv.tile([D + 1, 1024], F32, tag="pv")
                    for t in range(NT):
                        ps = psa.tile([128, 1024], F32, tag="ps")
                        for nn in (0, 512):
                            nc.tensor.matmul(
                                ps[:, nn : nn + 512],
                                kT_sb[0:D, h, t * 128 : (t + 1) * 128],
                                qT_sb[:, h, q0 + nn : q0 + nn + 512],
                                start=True,
                                stop=True,
                            )
                        pt = wpool.tile([128, 1024], F32, tag="pt")
                        nc.scalar.activation(
                            pt, ps, Exp, bias=kb_sb[:, t : t + 1], scale=SCALE
                        )
                        # zero the (key j = q+1) diagonal band
                        if t == 0:
                            if half == 0:
                                nc.vector.tensor_mul(
                                    pt[:, 0:127], pt[:, 0:127], dm0_sb[:, 0:127]
                                )
                        else:
                            w0 = t * 128 - 1
                            a = max(w0, q0)
                            b = min(w0 + 128, q0 + 1024, S)
                            if a < b:
                                nc.vector.tensor_mul(
                                    pt[:, a - q0 : b - q0],
                                    pt[:, a - q0 : b - q0],
                                    dmg_sb[:, a - w0 : b - w0],
                                )
                        for nn in (0, 512):
                            nc.tensor.matmul(
                                pv[:, nn : nn + 512],
                                v_sb[:, t, h, :],
                                pt[:, nn : nn + 512],
                                start=(t == 0),
                                stop=(t == NT - 1),
                            )
                    ov = opool.tile([D + 1, 1024], F32, tag="ov")
                    nc.vector.tensor_copy(ov, pv)
                    nc.sync.dma_start(
                        out=out_d[h, :, q0 : q0 + 1024], in_=ov
                    )

    nc.finalize()
    return nc


_NC = None


def _get_nc():
    global _NC
    if _NC is None:
        _NC = _build_nc()
    return _NC


def _host_prep(hidden_states, embx, expanded_embx, Wkv_w, Wkv_b,
               attention_mask, mlm_mask):
    hs = np.ascontiguousarray(np.asarray(hidden_states, np.float32))
    ex = np.ascontiguousarray(np.asarray(embx, np.float32))
    qx = np.asarray(expanded_embx, np.float32)
    w = np.asarray(Wkv_w, np.float32)
    bb = np.asarray(Wkv_b, np.float32)
    am = np.asarray(attention_mask).astype(bool)
    mm = np.asarray(mlm_mask).astype(bool)

    valid = am & ~mm                                   # (B, S)
    kbf = np.full((B, SK), NEG, np.float32)
    kbf[:, 0] = 0.0
    kbf[:, 1 : S + 1] = np.where(valid, 0.0, NEG)

    x = np.concatenate([ex, hs], axis=1)               # (B, S+1, HID)
    xT = np.zeros((B, HID, SK), np.float32)
    xT[:, :, : S + 1] = x.transpose(0, 2, 1)

    dm0 = np.ones((128, 128), np.float32)
    idx = np.arange(1, 128)
    dm0[idx, idx - 1] = 0.0
    dmg = np.ones((128, 128), np.float32)
    np.fill_diagonal(dmg, 0.0)

    in_maps = []
    for c in range(NCORE):
        b, g = divmod(c, 4)
        k_cols = slice(192 * g, 192 * g + 192)
        v_cols = slice(768 + 192 * g, 768 + 192 * g + 192)
        wg = np.concatenate([w[:, k_cols], w[:, v_cols]], axis=1)
        wg = np.ascontiguousarray(wg)                  # (768, 384)
        bk = np.ascontiguousarray(bb[k_cols].reshape(G, D).T)   # (64, 3)
        bv = np.ascontiguousarray(
            np.broadcast_to(bb[v_cols], (128, G * D))
        )
        qtg = np.ascontiguousarray(qx[b][:, k_cols].T)          # (192, 2048)
        kbt = np.ascontiguousarray(kbf[b].reshape(NT, 128).T)   # (128, 17)
        in_maps.append(
            dict(xT=np.ascontiguousarray(xT[b]), qT=qtg, W=wg, bk=bk,
                 bv=bv, kb=kbt, dm0=dm0, dmG=dmg)
        )
    return in_maps


def _host_post(results):
    out = np.empty((B, S, HID), np.float32)
    for c in range(NCORE):
        b, g = divmod(c, 4)
        ot = results[c]["outT"]                        # (3, 65, 2048)
        o = ot[:, :D, :] / ot[:, D : D + 1, :]         # (3, 64, 2048)
        out[b, :, 192 * g : 192 * g + 192] = (
            o.transpose(2, 0, 1).reshape(S, G * D)
        )
    return out


def kernel(hidden_states, embx, expanded_embx, Wkv_w, Wkv_b,
           attention_mask, mlm_mask):
    in_maps = _host_prep(hidden_states, embx, expanded_embx, Wkv_w, Wkv_b,
                         attention_mask, mlm_mask)
    nc = _get_nc()
    res = run_bass_kernel_spmd(nc, in_maps, list(range(NCORE)))
    return _host_post(res.results)
